# revision 2
# baseline (speedup 1.0000x reference)
"""AreaAttention Trainium2 kernel v3 (8 NeuronCores, batch-parallel).

Per 16-row tile (PT=4096 pixels, ST=1024 sites/dir):
  - k64 [64, PT]: rows 0:32 = k-replicas x4 pixel-order (dir0),
    rows 32:64 = k-replicas j-site-order (dir1). No bias (bk cancels in
    softmax over j; bq enters via q only).
  - q64 [64, ST]: rows 0:32 = q32-h (i-gathered, dir0), 32:64 = q32-v;
    built by PSUM-accumulated matmuls on i-sliced x views; one ACT copy
    adds bq.
  - scores: per (dir, j) one [32, ST] bf16 2x mul -> prod[32j+8i+c].
  - softmax: A16 = Osel@prod; E=exp(A); D=Osum@E; L=Ln(D);
    Ash=A-L (GPS); P=exp(Ash). No reciprocal.
  - apply: pair-broadcast (i, jp) OdP matmuls -> psum pb [128, 512];
    muls pb x V2{h,v} (contiguous-run views); bf16 in-place 2x adds.
  - combine on PE: o_psum[64, 512] = x + accH + accV via identity
    matmuls (accV via per-jc strided psum dst); DMA out from PSUM.
"""

import sys
import numpy as np

for _p in ("/opt/trn_rl_repo", "/root/.axon_site/_ro/trn_rl_repo"):
    if _p not in sys.path:
        sys.path.insert(0, _p)

from contextlib import ExitStack

from concourse import bass, bacc, tile, mybir
from concourse.bass_utils import run_bass_kernel_spmd

F32 = mybir.dt.float32
F32R = mybir.dt.float32r
BF16 = mybir.dt.bfloat16
AFT = mybir.ActivationFunctionType

B, C, H, W = 8, 64, 256, 256
A_ = 4
N_CORES = 8

ROWS_T = 16
N_TILES = H // ROWS_T        # 16
PT = ROWS_T * W              # 4096
ST = PT // A_                # 1024
NB = W // A_                 # 64

_cache = {}
BISECT_STRIDED = False
BISECT_B64 = True


def _consts(Wq, bq, Wk, bk, Wv, bv, gamma):
    # k-replica stationary [64, 32]: 4 replicas of Wk.T
    WkR = np.zeros((C, 32), np.float32)
    for r in range(4):
        WkR[:, r * 8:(r + 1) * 8] = Wk.T
    # q32 stationary: slot i maps x@(pos i) -> partitions 8i+c
    Wq32 = np.zeros((C, 4 * 32), np.float32)
    for i in range(4):
        Wq32[:, i * 32 + i * 8: i * 32 + (i + 1) * 8] = Wq.T
    bq64 = np.zeros((64, 1), np.float32)
    for i in range(4):
        bq64[i * 8:(i + 1) * 8, 0] = bq
        bq64[32 + i * 8:32 + (i + 1) * 8, 0] = bq

    g = np.float32(gamma[0])
    WvD = np.zeros((C, 128), np.float32)
    WvD[:, 0:64] = g * Wv.T
    WvD[:, 64:128] = g * Wv.T
    bvD = np.zeros((128, 1), np.float32)
    bvD[0:64, 0] = g * bv
    bvD[64:128, 0] = g * bv

    # prod row p = 32*j + 8*i + c  ->  pair col i*4+j
    Osel = np.zeros((128, 16), np.float32)
    for p in range(128):
        j, i, c = p >> 5, (p >> 3) & 3, p & 7
        Osel[p, i * 4 + j] = 1.0
    # D16 row (i,j) = sum_j' E[(i,j')]
    Osum = np.zeros((16, 16), np.float32)
    for p in range(16):
        for p2 in range(16):
            if p >> 2 == p2 >> 2:
                Osum[p, p2] = 1.0
    # paired broadcast: slot s=i*2+jp: cols 0:64 -> pair (i,jp), 64:128 ->
    # pair (i,jp+2)
    OdP = np.zeros((16, 8 * 128), np.float32)
    for i in range(4):
        for jp in range(2):
            s = i * 2 + jp
            OdP[i * 4 + jp, s * 128:s * 128 + 64] = 1.0
            OdP[i * 4 + jp + 2, s * 128 + 64:s * 128 + 128] = 1.0

    Id64 = np.eye(64, dtype=np.float32)
    return WkR, Wq32, bq64, WvD, bvD, Osel, Osum, OdP, Id64


def _build():
    nc = bacc.Bacc("TRN2", target_bir_lowering=False, debug=False,
                   num_devices=N_CORES)
    x_d = nc.dram_tensor("x", [C, H, W], F32, kind="ExternalInput")
    WkR_d = nc.dram_tensor("WkR", [C, 32], F32, kind="ExternalInput")
    Wq32_d = nc.dram_tensor("Wq32", [C, 128], F32, kind="ExternalInput")
    bq64_d = nc.dram_tensor("bq64", [64, 1], F32, kind="ExternalInput")
    WvD_d = nc.dram_tensor("WvD", [C, 128], F32, kind="ExternalInput")
    bvD_d = nc.dram_tensor("bvD", [128, 1], F32, kind="ExternalInput")
    Osel_d = nc.dram_tensor("Osel", [128, 16], F32, kind="ExternalInput")
    Osum_d = nc.dram_tensor("Osum", [16, 16], F32, kind="ExternalInput")
    OdP_d = nc.dram_tensor("OdP", [16, 8 * 128], F32, kind="ExternalInput")
    Id64_d = nc.dram_tensor("Id64", [64, 64], F32, kind="ExternalInput")
    out_d = nc.dram_tensor("out", [C, H, W], F32, kind="ExternalOutput")

    with tile.TileContext(nc) as tc, ExitStack() as ctx:
        consts = ctx.enter_context(tc.tile_pool(name="consts", bufs=1))
        xp = ctx.enter_context(tc.tile_pool(name="xp", bufs=2))
        qkp = ctx.enter_context(tc.tile_pool(name="qkp", bufs=2))
        v2p = ctx.enter_context(tc.tile_pool(name="v2p", bufs=2))
        accp = ctx.enter_context(tc.tile_pool(name="accp", bufs=2))
        smallp = ctx.enter_context(tc.tile_pool(name="smallp", bufs=3))
        prodp = ctx.enter_context(tc.tile_pool(name="prodp", bufs=2))
        p2p = ctx.enter_context(tc.tile_pool(name="p2p", bufs=4))
        # PSUM budget (8 banks): proj 2, pb 2, a 2, q 1, o 1
        ps_pp = ctx.enter_context(
            tc.tile_pool(name="ps_pp", bufs=2, space="PSUM"))
        ps_pb = ctx.enter_context(
            tc.tile_pool(name="ps_pb", bufs=2, space="PSUM"))
        ps_a = ctx.enter_context(
            tc.tile_pool(name="ps_a", bufs=2, space="PSUM"))
        ps_q = ctx.enter_context(
            tc.tile_pool(name="ps_q", bufs=1, space="PSUM"))
        ps_o = ctx.enter_context(
            tc.tile_pool(name="ps_o", bufs=1, space="PSUM"))

        WkR_s = consts.tile([C, 32], F32R)
        Wq32_s = consts.tile([C, 128], F32R)
        bq64_s = consts.tile([64, 1], F32)
        WvD_s = consts.tile([C, 128], F32R)
        bvD_s = consts.tile([128, 1], F32)
        Osel_s = consts.tile([128, 16], BF16)
        Osum_s = consts.tile([16, 16], BF16)
        OdP_s = consts.tile([16, 8 * 128], BF16)
        IdR_s = consts.tile([64, 64], F32R)
        IdLo_s = consts.tile([128, 64], BF16)
        IdHi_s = consts.tile([128, 64], BF16)
        for t, d in ((WkR_s, WkR_d), (Wq32_s, Wq32_d), (WvD_s, WvD_d),
                     (IdR_s, Id64_d)):
            nc.sync.dma_start(t[:], d[:].bitcast(F32R))
        for t, d in ((bq64_s, bq64_d), (bvD_s, bvD_d)):
            nc.sync.dma_start(t[:], d[:])
        Osel_f = consts.tile([128, 16], F32)
        Osum_f = consts.tile([16, 16], F32)
        OdP_f = consts.tile([16, 8 * 128], F32)
        Id_f = consts.tile([64, 64], F32)
        nc.sync.dma_start(Osel_f[:], Osel_d[:])
        nc.sync.dma_start(Osum_f[:], Osum_d[:])
        nc.sync.dma_start(OdP_f[:], OdP_d[:])
        nc.sync.dma_start(Id_f[:], Id64_d[:])
        nc.scalar.copy(Osel_s[:], Osel_f[:])
        nc.scalar.copy(Osum_s[:], Osum_f[:])
        nc.scalar.copy(OdP_s[:], OdP_f[:])
        nc.gpsimd.memset(IdLo_s[:], 0.0)
        nc.gpsimd.memset(IdHi_s[:], 0.0)
        nc.scalar.copy(IdLo_s[0:64, :], Id_f[:])
        nc.scalar.copy(IdHi_s[64:128, :], Id_f[:])

        for it in range(N_TILES):
            r0 = it * ROWS_T
            x_t = xp.tile([C, PT], F32R, tag="x")
            nc.sync.dma_start(x_t[:], x_d[:, r0:r0 + ROWS_T, :].bitcast(F32R))
            xr = x_t[:, :]
            x_hv = xr.rearrange("p (r w) -> p r w", w=W)            # rows
            x_vv = xr.rearrange("p (h nb jc) -> p h nb jc", nb=NB, jc=A_)

            # ---- k64 [64, PT]: k-h reps (0:32) + k-v reps (32:64) ----
            k64 = qkp.tile([64, PT], BF16, tag="k64")
            for m in range(PT // 512):
                ps = ps_pp.tile([128, 512], F32, tag="pp")
                nc.tensor.matmul(ps[0:32, :], WkR_s[:],
                                 xr[:, m * 512:(m + 1) * 512],
                                 start=True, stop=True)
                jc, hh = m // 2, m % 2
                mvv = x_vv[:, hh * 8:hh * 8 + 8, :, jc]
                ps2 = ps_pp.tile([128, 512], F32, tag="pp")
                nc.tensor.matmul(ps2[0:32, :], WkR_s[:], mvv,
                                 start=True, stop=True)
                sl = slice(m * 512, (m + 1) * 512)
                nc.scalar.copy(k64[0:32, sl], ps[0:32, :])
                nc.scalar.copy(k64[32:64, sl], ps2[0:32, :])

            # ---- q64 [64, ST]: q32-h (0:32) + q32-v (32:64), +bq ----
            q64s = []
            for half in range(2):
                qpsH = ps_q.tile([32, 512], F32, tag="q")
                qpsV = ps_q.tile([32, 512], F32, tag="q")
                q64s.append((qpsH, qpsV))
                for i in range(A_):
                    mv = x_hv[:, 8 * half + i:8 * half + i + 5:4, :]
                    nc.tensor.matmul(qpsH[:, :],
                                     Wq32_s[:, i * 32:(i + 1) * 32],
                                     mv, start=(i == 0), stop=(i == 3))
                for i in range(A_):
                    mv = x_vv[:, half * 8:half * 8 + 8, :, i]
                    nc.tensor.matmul(qpsV[:, :],
                                     Wq32_s[:, i * 32:(i + 1) * 32],
                                     mv, start=(i == 0), stop=(i == 3))
            q64 = qkp.tile([64, ST], BF16, tag="q64")
            for half in range(2):
                sl = slice(half * 512, (half + 1) * 512)
                nc.scalar.activation(q64[0:32, sl], q64s[half][0][:, :],
                                     AFT.Identity, bias=bq64_s[0:32, :])
                nc.scalar.activation(q64[32:64, sl], q64s[half][1][:, :],
                                     AFT.Identity, bias=bq64_s[32:64, :])

            # ---- V2h/V2v [128, PT] dup halves, gamma+bias folded ----
            V2h = v2p.tile([128, PT], BF16, tag="V2h")
            V2v = v2p.tile([128, PT], BF16, tag="V2v")
            for m in range(PT // 512):
                ps = ps_pp.tile([128, 512], F32, tag="pp")
                nc.tensor.matmul(ps[:], WvD_s[:],
                                 xr[:, m * 512:(m + 1) * 512],
                                 start=True, stop=True)
                nc.scalar.activation(V2h[:, m * 512:(m + 1) * 512], ps[:],
                                     AFT.Identity, bias=bvD_s[:])
            for m in range(PT // 512):
                ps = ps_pp.tile([128, 512], F32, tag="pp")
                jc, hh = m // 2, m % 2
                nc.tensor.matmul(ps[:], WvD_s[:],
                                 x_vv[:, hh * 8:hh * 8 + 8, :, jc],
                                 start=True, stop=True)
                nc.scalar.activation(V2v[:, m * 512:(m + 1) * 512], ps[:],
                                     AFT.Identity, bias=bvD_s[:])

            accs = []
            for d in range(2):
                # ---- scores ----
                prod = prodp.tile([128, ST], BF16, tag=f"prod{d}")
                if d == 0:
                    ksv = k64[0:32, :].rearrange("p (n i w) -> p n i w",
                                                 i=A_, w=W)
                    qsl = q64[0:32, :]
                else:
                    ksv = k64[32:64, :].rearrange("p (j s) -> p j s", j=A_)
                    qsl = q64[32:64, :]
                for j in range(A_):
                    dst = prod[32 * j:32 * (j + 1), :]
                    if d == 0:
                        ks = ksv[:, :, j, :]
                        dv = dst.rearrange("p (n w) -> p n w", w=W)
                        qv_ = qsl.rearrange("p (n w) -> p n w", w=W)
                        nc.vector.tensor_mul(dv, qv_, ks)
                    else:
                        ks = ksv[:, j, :]
                        nc.vector.tensor_mul(dst, qsl, ks)

                # ---- softmax ----
                P16 = smallp.tile([16, ST], BF16, tag="P16")
                for chk in range(2):
                    s0 = chk * 512
                    a_ps = ps_a.tile([16, 512], F32, tag="a")
                    nc.tensor.matmul(a_ps[:], Osel_s[:], prod[:, s0:s0 + 512],
                                     start=True, stop=True)
                    E = smallp.tile([16, 512], BF16, tag="E")
                    nc.scalar.activation(E[:], a_ps[:], AFT.Exp)
                    d_ps = ps_a.tile([16, 512], F32, tag="a")
                    nc.tensor.matmul(d_ps[:], Osum_s[:], E[:],
                                     start=True, stop=True)
                    R = smallp.tile([16, 512], F32, tag="R")
                    nc.vector.reciprocal_approx_fast(R[:], d_ps[:])
                    nc.vector.tensor_mul(P16[:, s0:s0 + 512], E[:], R[:])

                # ---- apply ----
                acc = accp.tile([128, 2 * ST], BF16, tag=f"acc{d}")
                accs.append(acc)
                Vt = V2h if d == 0 else V2v
                if d == 0:
                    vsv = Vt[:, :].rearrange("p (n i w) -> p n i w",
                                             i=A_, w=W)
                else:
                    vsv = Vt[:, :].rearrange("p (j s) -> p j s", j=A_)
                for i in range(A_):
                    for jp in range(2):
                        s_ = i * 2 + jp
                        for chk in range(2):
                            s0 = chk * 512
                            pb = ps_pb.tile([128, 512], F32, tag="pb")
                            nc.tensor.matmul(
                                pb[:], OdP_s[:, s_ * 128:(s_ + 1) * 128],
                                P16[:, s0:s0 + 512], start=True, stop=True)
                            dst = acc[:, jp * ST + s0:jp * ST + s0 + 512]
                            if d == 0:
                                # chunk = 2 n-blocks x 256 w
                                vi = vsv[:, 2 * chk:2 * chk + 2, i, :]
                                pbv = pb[:, :].rearrange(
                                    "p (n w) -> p n w", w=W)
                                dv = dst.rearrange("p (n w) -> p n w", w=W)
                            else:
                                vi = vsv[:, i, s0:s0 + 512]
                                pbv = pb[:, :]
                                dv = dst
                            if i == 0:
                                nc.vector.tensor_mul(dv, pbv, vi)
                            else:
                                p2 = p2p.tile([128, 512], BF16, tag="p2")
                                p2v = (p2[:, :].rearrange(
                                    "p (n w) -> p n w", w=W) if d == 0
                                    else p2[:, :])
                                nc.vector.tensor_mul(p2v, pbv, vi)
                                eng = nc.gpsimd if i == 1 else nc.vector
                                eng.tensor_add(dst, dst, p2[:, :])

            # ---- combine: PE sums accH+accV -> psum; x added on the
            # psum->sbuf hop; DMA out. 512-pix chunks = 2 rows (same jr
            # half-pair, so one aH matmul each).
            accH, accV = accs
            aH = accH[:, :].rearrange("p (e n w) -> p e n w", e=2, w=W)
            aV = accV[:, :].rearrange("p (e h nb) -> p e h nb", e=2, nb=NB)
            o_t = xp.tile([64, PT], F32, tag="ot")
            for oc in range(PT // 512):
                o_ps = ps_o.tile([64, 512], F32, tag="o")
                rbase = 2 * oc
                jr0 = rbase % 4
                hb = 64 * (jr0 >> 1)
                n = rbase // 4
                stH = IdLo_s if hb == 0 else IdHi_s
                nc.tensor.matmul(o_ps[:], stH[:],
                                 aH[:, :, n, :],
                                 start=True, stop=False)
                ojc = o_ps[:, :].rearrange("p (r nb jc) -> p r nb jc",
                                           nb=NB, jc=A_)
                for jc in range(A_):
                    stV = IdLo_s if jc < 2 else IdHi_s
                    src = aV[:, jc & 1, rbase:rbase + 2, :]
                    nc.tensor.matmul(ojc[:, :, :, jc], stV[:],
                                     src, start=False, stop=(jc == A_ - 1))
                osl = o_t[:, oc * 512:(oc + 1) * 512]
                nc.vector.tensor_add(osl, o_ps[:],
                                     xr[:, oc * 512:(oc + 1) * 512]
                                     .bitcast(F32))
            nc.scalar.dma_start(out_d[:, r0:r0 + ROWS_T, :], o_t[:])

    nc.compile()
    return nc


def _run(x, Wq, bq, Wk, bk, Wv, bv, gamma, **spmd_kwargs):
    x = np.asarray(x, np.float32)
    WkR, Wq32, bq64, WvD, bvD, Osel, Osum, OdP, Id64 = _consts(
        np.asarray(Wq, np.float32), np.asarray(bq, np.float32),
        np.asarray(Wk, np.float32), np.asarray(bk, np.float32),
        np.asarray(Wv, np.float32), np.asarray(bv, np.float32),
        np.asarray(gamma, np.float32))

    if "nc" not in _cache:
        _cache["nc"] = _build()
    nc = _cache["nc"]

    in_maps = []
    for b in range(N_CORES):
        in_maps.append({"x": np.ascontiguousarray(x[b]), "WkR": WkR,
                        "Wq32": Wq32, "bq64": bq64, "WvD": WvD, "bvD": bvD,
                        "Osel": Osel, "Osum": Osum, "OdP": OdP, "Id64": Id64})
    res = run_bass_kernel_spmd(nc, in_maps, core_ids=list(range(N_CORES)),
                               **spmd_kwargs)
    out = np.stack([res.results[b]["out"] for b in range(N_CORES)], axis=0)
    return out, res


def kernel(x, Wq, bq, Wk, bk, Wv, bv, gamma):
    return _run(x, Wq, bq, Wk, bk, Wv, bv, gamma)[0]


# revision 4
# speedup vs baseline: 1.2846x; 1.2846x over previous
"""AreaAttention Trainium2 kernel v3 (8 NeuronCores, batch-parallel).

Per 16-row tile (PT=4096 pixels, ST=1024 sites/dir):
  - k64 [64, PT]: rows 0:32 = k-replicas x4 pixel-order (dir0),
    rows 32:64 = k-replicas j-site-order (dir1). No bias (bk cancels in
    softmax over j; bq enters via q only).
  - q64 [64, ST]: rows 0:32 = q32-h (i-gathered, dir0), 32:64 = q32-v;
    built by PSUM-accumulated matmuls on i-sliced x views; one ACT copy
    adds bq.
  - scores: per (dir, j) one [32, ST] bf16 2x mul -> prod[32j+8i+c].
  - softmax: A16 = Osel@prod; E=exp(A); D=Osum@E; L=Ln(D);
    Ash=A-L (GPS); P=exp(Ash). No reciprocal.
  - apply: pair-broadcast (i, jp) OdP matmuls -> psum pb [128, 512];
    muls pb x V2{h,v} (contiguous-run views); bf16 in-place 2x adds.
  - combine on PE: o_psum[64, 512] = x + accH + accV via identity
    matmuls (accV via per-jc strided psum dst); DMA out from PSUM.
"""

import sys
import numpy as np

for _p in ("/opt/trn_rl_repo", "/root/.axon_site/_ro/trn_rl_repo"):
    if _p not in sys.path:
        sys.path.insert(0, _p)

from contextlib import ExitStack

from concourse import bass, bacc, tile, mybir
from concourse.bass_utils import run_bass_kernel_spmd

F32 = mybir.dt.float32
F32R = mybir.dt.float32r
BF16 = mybir.dt.bfloat16
AFT = mybir.ActivationFunctionType

B, C, H, W = 8, 64, 256, 256
A_ = 4
N_CORES = 8

ROWS_T = 16
N_TILES = H // ROWS_T        # 16
PT = ROWS_T * W              # 4096
ST = PT // A_                # 1024
NB = W // A_                 # 64

_cache = {}
BISECT_STRIDED = False
BISECT_B64 = True


def _consts(Wq, bq, Wk, bk, Wv, bv, gamma):
    # k-replica stationary + q8: cols 0:32 = 4x Wk.T, 32:40 = Wq.T
    WkR = np.zeros((C, 40), np.float32)
    for r in range(4):
        WkR[:, r * 8:(r + 1) * 8] = Wk.T
    WkR[:, 32:40] = Wq.T
    # q32 stationary: slot i maps x@(pos i) -> partitions 8i+c
    Wq32 = np.zeros((C, 4 * 32), np.float32)
    for i in range(4):
        Wq32[:, i * 32 + i * 8: i * 32 + (i + 1) * 8] = Wq.T
    bq64 = np.zeros((64, 1), np.float32)
    for i in range(4):
        bq64[i * 8:(i + 1) * 8, 0] = bq
        bq64[32 + i * 8:32 + (i + 1) * 8, 0] = bq

    g = np.float32(gamma[0])
    WvD = np.zeros((C, 128), np.float32)
    WvD[:, 0:64] = g * Wv.T
    WvD[:, 64:128] = g * Wv.T
    bvD = np.zeros((128, 1), np.float32)
    bvD[0:64, 0] = g * bv
    bvD[64:128, 0] = g * bv

    # prod row p = 32*j + 8*i + c  ->  pair col i*4+j
    Osel = np.zeros((128, 16), np.float32)
    for p in range(128):
        j, i, c = p >> 5, (p >> 3) & 3, p & 7
        Osel[p, i * 4 + j] = 1.0
    # D16 row (i,j) = sum_j' E[(i,j')]
    Osum = np.zeros((16, 16), np.float32)
    for p in range(16):
        for p2 in range(16):
            if p >> 2 == p2 >> 2:
                Osum[p, p2] = 1.0
    # paired broadcast: slot s=i*2+jp: cols 0:64 -> pair (i,jp), 64:128 ->
    # pair (i,jp+2)
    OdP = np.zeros((16, 8 * 128), np.float32)
    for i in range(4):
        for jp in range(2):
            s = i * 2 + jp
            OdP[i * 4 + jp, s * 128:s * 128 + 64] = 1.0
            OdP[i * 4 + jp + 2, s * 128 + 64:s * 128 + 128] = 1.0

    Id64 = np.eye(64, dtype=np.float32)
    return WkR, Wq32, bq64, WvD, bvD, Osel, Osum, OdP, Id64


def _build():
    nc = bacc.Bacc("TRN2", target_bir_lowering=False, debug=False,
                   num_devices=N_CORES)
    x_d = nc.dram_tensor("x", [C, H, W], F32, kind="ExternalInput")
    WkR_d = nc.dram_tensor("WkR", [C, 40], F32, kind="ExternalInput")
    Wq32_d = nc.dram_tensor("Wq32", [C, 128], F32, kind="ExternalInput")
    bq64_d = nc.dram_tensor("bq64", [64, 1], F32, kind="ExternalInput")
    WvD_d = nc.dram_tensor("WvD", [C, 128], F32, kind="ExternalInput")
    bvD_d = nc.dram_tensor("bvD", [128, 1], F32, kind="ExternalInput")
    Osel_d = nc.dram_tensor("Osel", [128, 16], F32, kind="ExternalInput")
    Osum_d = nc.dram_tensor("Osum", [16, 16], F32, kind="ExternalInput")
    OdP_d = nc.dram_tensor("OdP", [16, 8 * 128], F32, kind="ExternalInput")
    Id64_d = nc.dram_tensor("Id64", [64, 64], F32, kind="ExternalInput")
    out_d = nc.dram_tensor("out", [C, H, W], F32, kind="ExternalOutput")

    with tile.TileContext(nc) as tc, ExitStack() as ctx:
        consts = ctx.enter_context(tc.tile_pool(name="consts", bufs=1))
        xp = ctx.enter_context(tc.tile_pool(name="xp", bufs=2))
        qkp = ctx.enter_context(tc.tile_pool(name="qkp", bufs=2))
        v2p = ctx.enter_context(tc.tile_pool(name="v2p", bufs=2))
        accp = ctx.enter_context(tc.tile_pool(name="accp", bufs=2))
        smallp = ctx.enter_context(tc.tile_pool(name="smallp", bufs=3))
        prodp = ctx.enter_context(tc.tile_pool(name="prodp", bufs=2))
        p2p = ctx.enter_context(tc.tile_pool(name="p2p", bufs=4))
        # PSUM budget (8 banks): proj 2, pb 2, a 2, q 1, o 1
        ps_pp = ctx.enter_context(
            tc.tile_pool(name="ps_pp", bufs=2, space="PSUM"))
        ps_pb = ctx.enter_context(
            tc.tile_pool(name="ps_pb", bufs=2, space="PSUM"))
        ps_a = ctx.enter_context(
            tc.tile_pool(name="ps_a", bufs=1, space="PSUM"))
        ps_q = ctx.enter_context(
            tc.tile_pool(name="ps_q", bufs=1, space="PSUM"))
        ps_o = ctx.enter_context(
            tc.tile_pool(name="ps_o", bufs=2, space="PSUM"))

        WkR_s = consts.tile([C, 40], F32R)
        Wq32_s = consts.tile([C, 128], F32R)
        bq64_s = consts.tile([64, 1], F32)
        bq8_s = consts.tile([8, 1], F32)
        WvD_s = consts.tile([C, 128], F32R)
        bvD_s = consts.tile([128, 1], F32)
        Osel_s = consts.tile([128, 16], BF16)
        Osum_s = consts.tile([16, 16], BF16)
        OdP_s = consts.tile([16, 8 * 128], BF16)
        IdR_s = consts.tile([64, 64], F32R)
        IdLo_s = consts.tile([128, 64], BF16)
        IdHi_s = consts.tile([128, 64], BF16)
        for t, d in ((WkR_s, WkR_d), (Wq32_s, Wq32_d), (WvD_s, WvD_d),
                     (IdR_s, Id64_d)):
            nc.sync.dma_start(t[:], d[:].bitcast(F32R))
        for t, d in ((bq64_s, bq64_d), (bvD_s, bvD_d)):
            nc.sync.dma_start(t[:], d[:])
        Osel_f = consts.tile([128, 16], F32)
        Osum_f = consts.tile([16, 16], F32)
        OdP_f = consts.tile([16, 8 * 128], F32)
        Id_f = consts.tile([64, 64], F32)
        nc.sync.dma_start(Osel_f[:], Osel_d[:])
        nc.sync.dma_start(Osum_f[:], Osum_d[:])
        nc.sync.dma_start(OdP_f[:], OdP_d[:])
        nc.sync.dma_start(Id_f[:], Id64_d[:])
        nc.scalar.copy(Osel_s[:], Osel_f[:])
        nc.scalar.copy(Osum_s[:], Osum_f[:])
        nc.scalar.copy(OdP_s[:], OdP_f[:])
        nc.scalar.copy(bq8_s[:], bq64_s[0:8, :])
        nc.gpsimd.memset(IdLo_s[:], 0.0)
        nc.gpsimd.memset(IdHi_s[:], 0.0)
        nc.scalar.copy(IdLo_s[0:64, :], Id_f[:])
        nc.scalar.copy(IdHi_s[64:128, :], Id_f[:])

        for it in range(N_TILES):
            r0 = it * ROWS_T
            x_t = xp.tile([C, PT], F32R, tag="x")
            nc.sync.dma_start(x_t[:], x_d[:, r0:r0 + ROWS_T, :].bitcast(F32R))
            xr = x_t[:, :]
            x_hv = xr.rearrange("p (r w) -> p r w", w=W)            # rows
            x_vv = xr.rearrange("p (h nb jc) -> p h nb jc", nb=NB, jc=A_)

            # ---- k64 [64, PT]: k-h reps (0:32) + k-v reps (32:64) ----
            k64 = qkp.tile([64, PT], BF16, tag="k64")
            q8 = qkp.tile([8, PT], BF16, tag="q8")
            for m in range(PT // 512):
                ps = ps_pp.tile([128, 512], F32, tag="pp")
                nc.tensor.matmul(ps[0:40, :], WkR_s[:],
                                 xr[:, m * 512:(m + 1) * 512],
                                 start=True, stop=True)
                nc.scalar.activation(q8[:, m * 512:(m + 1) * 512],
                                     ps[32:40, :], AFT.Identity,
                                     bias=bq8_s[:])
                jc, hh = m // 2, m % 2
                mvv = x_vv[:, hh * 8:hh * 8 + 8, :, jc]
                ps2 = ps_pp.tile([128, 512], F32, tag="pp")
                nc.tensor.matmul(ps2[0:32, :], WkR_s[:, 0:32], mvv,
                                 start=True, stop=True)
                sl = slice(m * 512, (m + 1) * 512)
                nc.scalar.copy(k64[0:32, sl], ps[0:32, :])
                nc.scalar.copy(k64[32:64, sl], ps2[0:32, :])

            # ---- q64 [64, ST]: q32-h (0:32) + q32-v (32:64), +bq ----
            q64 = qkp.tile([64, ST], BF16, tag="q64")
            q8v = q8[:, :].rearrange("p (r w) -> p r w", w=W)
            q64s = []
            for half in range(2):
                # dir0: DMA-gather q8 rows {8h+i, 8h+i+4} -> q64[8i:8i+8]
                for i in range(A_):
                    src = q8v[:, 8 * half + i:8 * half + i + 5:4, :]
                    nc.sync.dma_start(
                        q64[8 * i:8 * (i + 1),
                            half * 512:(half + 1) * 512], src)
                qpsV = ps_q.tile([32, 512], F32, tag="q")
                q64s.append(qpsV)
                for i in range(A_):
                    mv = x_vv[:, half * 8:half * 8 + 8, :, i]
                    nc.tensor.matmul(qpsV[:, :],
                                     Wq32_s[:, i * 32:(i + 1) * 32],
                                     mv, start=(i == 0), stop=(i == 3))
            for half in range(2):
                sl = slice(half * 512, (half + 1) * 512)
                nc.scalar.activation(q64[32:64, sl], q64s[half][:, :],
                                     AFT.Identity, bias=bq64_s[32:64, :])

            # ---- V2h/V2v [128, PT] dup halves, gamma+bias folded ----
            V2h = v2p.tile([128, PT], BF16, tag="V2h")
            V2v = v2p.tile([128, PT], BF16, tag="V2v")
            for m in range(PT // 512):
                ps = ps_pp.tile([128, 512], F32, tag="pp")
                nc.tensor.matmul(ps[:], WvD_s[:],
                                 xr[:, m * 512:(m + 1) * 512],
                                 start=True, stop=True)
                nc.scalar.activation(V2h[:, m * 512:(m + 1) * 512], ps[:],
                                     AFT.Identity, bias=bvD_s[:])
            for m in range(PT // 512):
                ps = ps_pp.tile([128, 512], F32, tag="pp")
                jc, hh = m // 2, m % 2
                nc.tensor.matmul(ps[:], WvD_s[:],
                                 x_vv[:, hh * 8:hh * 8 + 8, :, jc],
                                 start=True, stop=True)
                nc.scalar.activation(V2v[:, m * 512:(m + 1) * 512], ps[:],
                                     AFT.Identity, bias=bvD_s[:])

            accs = []
            for d in range(2):
                # ---- scores ----
                prod = prodp.tile([128, ST], BF16, tag=f"prod{d}")
                if d == 0:
                    ksv = k64[0:32, :].rearrange("p (n i w) -> p n i w",
                                                 i=A_, w=W)
                    qsl = q64[0:32, :]
                else:
                    ksv = k64[32:64, :].rearrange("p (j s) -> p j s", j=A_)
                    qsl = q64[32:64, :]
                for j in range(A_):
                    dst = prod[32 * j:32 * (j + 1), :]
                    if d == 0:
                        ks = ksv[:, :, j, :]
                        dv = dst.rearrange("p (n w) -> p n w", w=W)
                        qv_ = qsl.rearrange("p (n w) -> p n w", w=W)
                        nc.vector.tensor_mul(dv, qv_, ks)
                    else:
                        ks = ksv[:, j, :]
                        nc.vector.tensor_mul(dst, qsl, ks)

                # ---- softmax ----
                P16 = smallp.tile([16, ST], BF16, tag="P16")
                for chk in range(2):
                    s0 = chk * 512
                    a_ps = ps_a.tile([16, 512], F32, tag="a")
                    nc.tensor.matmul(a_ps[:], Osel_s[:], prod[:, s0:s0 + 512],
                                     start=True, stop=True)
                    E = smallp.tile([16, 512], BF16, tag="E")
                    nc.scalar.activation(E[:], a_ps[:], AFT.Exp)
                    d_ps = ps_a.tile([16, 512], F32, tag="a")
                    nc.tensor.matmul(d_ps[:], Osum_s[:], E[:],
                                     start=True, stop=True)
                    R = smallp.tile([16, 512], F32, tag="R")
                    nc.vector.reciprocal_approx_fast(R[:], d_ps[:])
                    nc.vector.tensor_mul(P16[:, s0:s0 + 512], E[:], R[:])

                # ---- apply ----
                acc = accp.tile([128, 2 * ST], BF16, tag=f"acc{d}")
                accs.append(acc)
                Vt = V2h if d == 0 else V2v
                if d == 0:
                    vsv = Vt[:, :].rearrange("p (n i w) -> p n i w",
                                             i=A_, w=W)
                else:
                    vsv = Vt[:, :].rearrange("p (j s) -> p j s", j=A_)
                for i in range(A_):
                    for jp in range(2):
                        s_ = i * 2 + jp
                        for chk in range(2):
                            s0 = chk * 512
                            pb = ps_pb.tile([128, 512], F32, tag="pb")
                            nc.tensor.matmul(
                                pb[:], OdP_s[:, s_ * 128:(s_ + 1) * 128],
                                P16[:, s0:s0 + 512], start=True, stop=True)
                            dst = acc[:, jp * ST + s0:jp * ST + s0 + 512]
                            if d == 0:
                                # chunk = 2 n-blocks x 256 w
                                vi = vsv[:, 2 * chk:2 * chk + 2, i, :]
                                pbv = pb[:, :].rearrange(
                                    "p (n w) -> p n w", w=W)
                                dv = dst.rearrange("p (n w) -> p n w", w=W)
                            else:
                                vi = vsv[:, i, s0:s0 + 512]
                                pbv = pb[:, :]
                                dv = dst
                            if i == 0:
                                nc.vector.tensor_mul(dv, pbv, vi)
                            else:
                                p2 = p2p.tile([128, 512], BF16, tag="p2")
                                p2v = (p2[:, :].rearrange(
                                    "p (n w) -> p n w", w=W) if d == 0
                                    else p2[:, :])
                                nc.vector.tensor_mul(p2v, pbv, vi)
                                eng = nc.gpsimd if i in (1, 2) else nc.vector
                                eng.tensor_add(dst, dst, p2[:, :])

            # ---- combine: PE sums accH+accV -> psum; x added on the
            # psum->sbuf hop; DMA out. 512-pix chunks = 2 rows (same jr
            # half-pair, so one aH matmul each).
            accH, accV = accs
            aH = accH[:, :].rearrange("p (e n w) -> p e n w", e=2, w=W)
            aV = accV[:, :].rearrange("p (e h nb) -> p e h nb", e=2, nb=NB)
            o_t = xp.tile([64, PT], F32, tag="ot")
            for oc in range(PT // 512):
                o_ps = ps_o.tile([64, 512], F32, tag="o")
                rbase = 2 * oc
                jr0 = rbase % 4
                hb = 64 * (jr0 >> 1)
                n = rbase // 4
                stH = IdLo_s if hb == 0 else IdHi_s
                nc.tensor.matmul(o_ps[:], stH[:],
                                 aH[:, :, n, :],
                                 start=True, stop=False)
                ojc = o_ps[:, :].rearrange("p (r nb jc) -> p r nb jc",
                                           nb=NB, jc=A_)
                for jc in range(A_):
                    stV = IdLo_s if jc < 2 else IdHi_s
                    src = aV[:, jc & 1, rbase:rbase + 2, :]
                    nc.tensor.matmul(ojc[:, :, :, jc], stV[:],
                                     src, start=False, stop=(jc == A_ - 1))
                osl = o_t[:, oc * 512:(oc + 1) * 512]
                nc.vector.tensor_add(osl, o_ps[:],
                                     xr[:, oc * 512:(oc + 1) * 512]
                                     .bitcast(F32))
            nc.scalar.dma_start(out_d[:, r0:r0 + ROWS_T, :], o_t[:])

    nc.compile()
    return nc


def _run(x, Wq, bq, Wk, bk, Wv, bv, gamma, **spmd_kwargs):
    x = np.asarray(x, np.float32)
    WkR, Wq32, bq64, WvD, bvD, Osel, Osum, OdP, Id64 = _consts(
        np.asarray(Wq, np.float32), np.asarray(bq, np.float32),
        np.asarray(Wk, np.float32), np.asarray(bk, np.float32),
        np.asarray(Wv, np.float32), np.asarray(bv, np.float32),
        np.asarray(gamma, np.float32))

    if "nc" not in _cache:
        _cache["nc"] = _build()
    nc = _cache["nc"]

    in_maps = []
    for b in range(N_CORES):
        in_maps.append({"x": np.ascontiguousarray(x[b]), "WkR": WkR,
                        "Wq32": Wq32, "bq64": bq64, "WvD": WvD, "bvD": bvD,
                        "Osel": Osel, "Osum": Osum, "OdP": OdP, "Id64": Id64})
    res = run_bass_kernel_spmd(nc, in_maps, core_ids=list(range(N_CORES)),
                               **spmd_kwargs)
    out = np.stack([res.results[b]["out"] for b in range(N_CORES)], axis=0)
    return out, res


def kernel(x, Wq, bq, Wk, bk, Wv, bv, gamma):
    return _run(x, Wq, bq, Wk, bk, Wv, bv, gamma)[0]


# revision 6
# speedup vs baseline: 1.3591x; 1.0580x over previous
"""AreaAttention Trainium2 kernel v3 (8 NeuronCores, batch-parallel).

Per 16-row tile (PT=4096 pixels, ST=1024 sites/dir):
  - k64 [64, PT]: rows 0:32 = k-replicas x4 pixel-order (dir0),
    rows 32:64 = k-replicas j-site-order (dir1). No bias (bk cancels in
    softmax over j; bq enters via q only).
  - q64 [64, ST]: rows 0:32 = q32-h (i-gathered, dir0), 32:64 = q32-v;
    built by PSUM-accumulated matmuls on i-sliced x views; one ACT copy
    adds bq.
  - scores: per (dir, j) one [32, ST] bf16 2x mul -> prod[32j+8i+c].
  - softmax: A16 = Osel@prod; E=exp(A); D=Osum@E; L=Ln(D);
    Ash=A-L (GPS); P=exp(Ash). No reciprocal.
  - apply: pair-broadcast (i, jp) OdP matmuls -> psum pb [128, 512];
    muls pb x V2{h,v} (contiguous-run views); bf16 in-place 2x adds.
  - combine on PE: o_psum[64, 512] = x + accH + accV via identity
    matmuls (accV via per-jc strided psum dst); DMA out from PSUM.
"""

import sys
import numpy as np

for _p in ("/opt/trn_rl_repo", "/root/.axon_site/_ro/trn_rl_repo"):
    if _p not in sys.path:
        sys.path.insert(0, _p)

from contextlib import ExitStack

from concourse import bass, bacc, tile, mybir
from concourse.bass_utils import run_bass_kernel_spmd

F32 = mybir.dt.float32
F32R = mybir.dt.float32r
BF16 = mybir.dt.bfloat16
AFT = mybir.ActivationFunctionType

B, C, H, W = 8, 64, 256, 256
A_ = 4
N_CORES = 8

ROWS_T = 16
N_TILES = H // ROWS_T        # 16
PT = ROWS_T * W              # 4096
ST = PT // A_                # 1024
NB = W // A_                 # 64

_cache = {}
BISECT_STRIDED = False
BISECT_B64 = True


def _consts(Wq, bq, Wk, bk, Wv, bv, gamma):
    # k-replica stationary + q8: cols 0:32 = 4x Wk.T, 32:40 = Wq.T
    WkR = np.zeros((C, 40), np.float32)
    for r in range(4):
        WkR[:, r * 8:(r + 1) * 8] = Wk.T
    WkR[:, 32:40] = Wq.T
    # q32 stationary: slot i maps x@(pos i) -> partitions 8i+c
    Wq32 = np.zeros((C, 4 * 32), np.float32)
    for i in range(4):
        Wq32[:, i * 32 + i * 8: i * 32 + (i + 1) * 8] = Wq.T
    bq64 = np.zeros((64, 1), np.float32)
    for i in range(4):
        bq64[i * 8:(i + 1) * 8, 0] = bq
        bq64[32 + i * 8:32 + (i + 1) * 8, 0] = bq

    g = np.float32(gamma[0])
    WvD = np.zeros((C, 128), np.float32)
    WvD[:, 0:64] = g * Wv.T
    WvD[:, 64:128] = g * Wv.T
    bvD = np.zeros((128, 1), np.float32)
    bvD[0:64, 0] = g * bv
    bvD[64:128, 0] = g * bv

    # prod row p = 32*j + 8*i + c  ->  pair col i*4+j
    Osel = np.zeros((128, 16), np.float32)
    for p in range(128):
        j, i, c = p >> 5, (p >> 3) & 3, p & 7
        Osel[p, i * 4 + j] = 1.0
    # D16 row (i,j) = sum_j' E[(i,j')]
    Osum = np.zeros((16, 16), np.float32)
    for p in range(16):
        for p2 in range(16):
            if p >> 2 == p2 >> 2:
                Osum[p, p2] = 1.0
    # paired broadcast: slot s=i*2+jp: cols 0:64 -> pair (i,jp), 64:128 ->
    # pair (i,jp+2). Tall [48,...]: even slots read P at rows 0:16, odd at
    # rows 32:48 (duplicated P) so pb LDWEIGHTS alternate row groups.
    OdP = np.zeros((48, 8 * 128), np.float32)
    for i in range(4):
        for jp in range(2):
            s = i * 2 + jp
            rb = 0 if s % 2 == 0 else 32
            OdP[rb + i * 4 + jp, s * 128:s * 128 + 64] = 1.0
            OdP[rb + i * 4 + jp + 2, s * 128 + 64:s * 128 + 128] = 1.0

    Id64 = np.eye(64, dtype=np.float32)
    return WkR, Wq32, bq64, WvD, bvD, Osel, Osum, OdP, Id64


def _build():
    nc = bacc.Bacc("TRN2", target_bir_lowering=False, debug=False,
                   num_devices=N_CORES)
    x_d = nc.dram_tensor("x", [C, H, W], F32, kind="ExternalInput")
    WkR_d = nc.dram_tensor("WkR", [C, 40], F32, kind="ExternalInput")
    Wq32_d = nc.dram_tensor("Wq32", [C, 128], F32, kind="ExternalInput")
    bq64_d = nc.dram_tensor("bq64", [64, 1], F32, kind="ExternalInput")
    WvD_d = nc.dram_tensor("WvD", [C, 128], F32, kind="ExternalInput")
    bvD_d = nc.dram_tensor("bvD", [128, 1], F32, kind="ExternalInput")
    Osel_d = nc.dram_tensor("Osel", [128, 16], F32, kind="ExternalInput")
    Osum_d = nc.dram_tensor("Osum", [16, 16], F32, kind="ExternalInput")
    OdP_d = nc.dram_tensor("OdP", [48, 8 * 128], F32, kind="ExternalInput")
    Id64_d = nc.dram_tensor("Id64", [64, 64], F32, kind="ExternalInput")
    out_d = nc.dram_tensor("out", [C, H, W], F32, kind="ExternalOutput")

    with tile.TileContext(nc) as tc, ExitStack() as ctx:
        consts = ctx.enter_context(tc.tile_pool(name="consts", bufs=1))
        xp = ctx.enter_context(tc.tile_pool(name="xp", bufs=2))
        qkp = ctx.enter_context(tc.tile_pool(name="qkp", bufs=2))
        v2p = ctx.enter_context(tc.tile_pool(name="v2p", bufs=2))
        accp = ctx.enter_context(tc.tile_pool(name="accp", bufs=2))
        smallp = ctx.enter_context(tc.tile_pool(name="smallp", bufs=3))
        prodp = ctx.enter_context(tc.tile_pool(name="prodp", bufs=2))
        p2p = ctx.enter_context(tc.tile_pool(name="p2p", bufs=4))
        # PSUM budget (8 banks): proj 2, pb 2, a 2, q 1, o 1
        ps_pp = ctx.enter_context(
            tc.tile_pool(name="ps_pp", bufs=2, space="PSUM"))
        ps_pb = ctx.enter_context(
            tc.tile_pool(name="ps_pb", bufs=2, space="PSUM"))
        ps_a = ctx.enter_context(
            tc.tile_pool(name="ps_a", bufs=2, space="PSUM"))
        ps_o = ctx.enter_context(
            tc.tile_pool(name="ps_o", bufs=2, space="PSUM"))

        WkR_s = consts.tile([C, 40], F32R)
        Wq32_s = consts.tile([C, 128], F32R)
        bq64_s = consts.tile([64, 1], F32)
        bq8_s = consts.tile([8, 1], F32)
        WvD_s = consts.tile([C, 128], F32R)
        bvD_s = consts.tile([128, 1], F32)
        Osel_s = consts.tile([128, 16], BF16)
        Osum_s = consts.tile([16, 16], BF16)
        OdP_s = consts.tile([48, 8 * 128], BF16)
        IdR_s = consts.tile([64, 64], F32R)
        IdLo_s = consts.tile([128, 64], BF16)
        IdHi_s = consts.tile([128, 64], BF16)
        for t, d in ((WkR_s, WkR_d), (Wq32_s, Wq32_d), (WvD_s, WvD_d),
                     (IdR_s, Id64_d)):
            nc.sync.dma_start(t[:], d[:].bitcast(F32R))
        for t, d in ((bq64_s, bq64_d), (bvD_s, bvD_d)):
            nc.sync.dma_start(t[:], d[:])
        Osel_f = consts.tile([128, 16], F32)
        Osum_f = consts.tile([16, 16], F32)
        OdP_f = consts.tile([48, 8 * 128], F32)
        Id_f = consts.tile([64, 64], F32)
        nc.sync.dma_start(Osel_f[:], Osel_d[:])
        nc.sync.dma_start(Osum_f[:], Osum_d[:])
        nc.sync.dma_start(OdP_f[:], OdP_d[:])
        nc.sync.dma_start(Id_f[:], Id64_d[:])
        nc.scalar.copy(Osel_s[:], Osel_f[:])
        nc.scalar.copy(Osum_s[:], Osum_f[:])
        nc.scalar.copy(OdP_s[:], OdP_f[:])
        nc.scalar.copy(bq8_s[:], bq64_s[0:8, :])
        nc.gpsimd.memset(IdLo_s[:], 0.0)
        nc.gpsimd.memset(IdHi_s[:], 0.0)
        nc.scalar.copy(IdLo_s[0:64, :], Id_f[:])
        nc.scalar.copy(IdHi_s[64:128, :], Id_f[:])

        for it in range(N_TILES):
            r0 = it * ROWS_T
            x_t = xp.tile([C, PT], F32R, tag="x")
            nc.sync.dma_start(x_t[:], x_d[:, r0:r0 + ROWS_T, :].bitcast(F32R))
            xr = x_t[:, :]
            x_hv = xr.rearrange("p (r w) -> p r w", w=W)            # rows
            x_vv = xr.rearrange("p (h nb jc) -> p h nb jc", nb=NB, jc=A_)

            # ---- k64 [64, PT]: k-h reps (0:32) + k-v reps (32:64) ----
            k64 = qkp.tile([64, PT], BF16, tag="k64")
            q8 = qkp.tile([8, PT], BF16, tag="q8")
            for m in range(PT // 512):
                ps = ps_pp.tile([128, 512], F32, tag="pp")
                nc.tensor.matmul(ps[0:40, :], WkR_s[:],
                                 xr[:, m * 512:(m + 1) * 512],
                                 start=True, stop=True)
                nc.scalar.activation(q8[:, m * 512:(m + 1) * 512],
                                     ps[32:40, :], AFT.Identity,
                                     bias=bq8_s[:])
                jc, hh = m // 2, m % 2
                mvv = x_vv[:, hh * 8:hh * 8 + 8, :, jc]
                ps2 = ps_pp.tile([128, 512], F32, tag="pp")
                nc.tensor.matmul(ps2[0:32, :], WkR_s[:, 0:32], mvv,
                                 start=True, stop=True)
                sl = slice(m * 512, (m + 1) * 512)
                nc.scalar.copy(k64[0:32, sl], ps[0:32, :])
                nc.scalar.copy(k64[32:64, sl], ps2[0:32, :])

            # ---- q64 [64, ST]: q32-h (0:32) + q32-v (32:64), +bq ----
            q64 = qkp.tile([64, ST], BF16, tag="q64")
            q8v = q8[:, :].rearrange("p (r w) -> p r w", w=W)
            q64s = []
            for half in range(2):
                # dir0: DMA-gather q8 rows {8h+i, 8h+i+4} -> q64[8i:8i+8]
                for i in range(A_):
                    src = q8v[:, 8 * half + i:8 * half + i + 5:4, :]
                    nc.gpsimd.dma_start(
                        q64[8 * i:8 * (i + 1),
                            half * 512:(half + 1) * 512], src)
                qpsV = ps_a.tile([32, 512], F32, tag="a")
                q64s.append(qpsV)
                for i in range(A_):
                    mv = x_vv[:, half * 8:half * 8 + 8, :, i]
                    nc.tensor.matmul(qpsV[:, :],
                                     Wq32_s[:, i * 32:(i + 1) * 32],
                                     mv, start=(i == 0), stop=(i == 3))
            for half in range(2):
                sl = slice(half * 512, (half + 1) * 512)
                nc.scalar.activation(q64[32:64, sl], q64s[half][:, :],
                                     AFT.Identity, bias=bq64_s[32:64, :])

            # ---- V2h/V2v [128, PT] dup halves, gamma+bias folded ----
            V2h = v2p.tile([128, PT], BF16, tag="V2h")
            V2v = v2p.tile([128, PT], BF16, tag="V2v")
            for m in range(PT // 512):
                ps = ps_pp.tile([128, 512], F32, tag="pp")
                nc.tensor.matmul(ps[:], WvD_s[:],
                                 xr[:, m * 512:(m + 1) * 512],
                                 start=True, stop=True)
                nc.scalar.activation(V2h[:, m * 512:(m + 1) * 512], ps[:],
                                     AFT.Identity, bias=bvD_s[:])
            for m in range(PT // 512):
                ps = ps_pp.tile([128, 512], F32, tag="pp")
                jc, hh = m // 2, m % 2
                nc.tensor.matmul(ps[:], WvD_s[:],
                                 x_vv[:, hh * 8:hh * 8 + 8, :, jc],
                                 start=True, stop=True)
                nc.scalar.activation(V2v[:, m * 512:(m + 1) * 512], ps[:],
                                     AFT.Identity, bias=bvD_s[:])

            accs = []
            for d in range(2):
                # ---- scores ----
                prod = prodp.tile([128, ST], BF16, tag=f"prod{d}")
                if d == 0:
                    ksv = k64[0:32, :].rearrange("p (n i w) -> p n i w",
                                                 i=A_, w=W)
                    qsl = q64[0:32, :]
                else:
                    ksv = k64[32:64, :].rearrange("p (j s) -> p j s", j=A_)
                    qsl = q64[32:64, :]
                for j in range(A_):
                    dst = prod[32 * j:32 * (j + 1), :]
                    if d == 0:
                        ks = ksv[:, :, j, :]
                        dv = dst.rearrange("p (n w) -> p n w", w=W)
                        qv_ = qsl.rearrange("p (n w) -> p n w", w=W)
                        nc.vector.tensor_mul(dv, qv_, ks)
                    else:
                        ks = ksv[:, j, :]
                        nc.vector.tensor_mul(dst, qsl, ks)

                # ---- softmax ----
                P16 = smallp.tile([48, ST], BF16, tag="P16")
                nc.gpsimd.memset(P16[0:32, :], 0.0)
                for chk in range(2):
                    s0 = chk * 512
                    a_ps = ps_a.tile([16, 512], F32, tag="a")
                    nc.tensor.matmul(a_ps[:], Osel_s[:], prod[:, s0:s0 + 512],
                                     start=True, stop=True)
                    E = smallp.tile([16, 512], BF16, tag="E")
                    nc.scalar.activation(E[:], a_ps[:], AFT.Exp)
                    d_ps = ps_a.tile([16, 512], F32, tag="a")
                    nc.tensor.matmul(d_ps[:], Osum_s[:], E[:],
                                     start=True, stop=True)
                    R = smallp.tile([16, 512], F32, tag="R")
                    nc.vector.reciprocal_approx_fast(R[:], d_ps[:])
                    nc.vector.tensor_mul(P16[0:16, s0:s0 + 512], E[:], R[:])
                    nc.vector.tensor_copy(P16[32:48, s0:s0 + 512],
                                          P16[0:16, s0:s0 + 512])

                # ---- apply ----
                acc = accp.tile([128, 2 * ST], BF16, tag=f"acc{d}")
                accs.append(acc)
                Vt = V2h if d == 0 else V2v
                if d == 0:
                    vsv = Vt[:, :].rearrange("p (n i w) -> p n i w",
                                             i=A_, w=W)
                else:
                    vsv = Vt[:, :].rearrange("p (j s) -> p j s", j=A_)
                for i in range(A_):
                    for jp in range(2):
                        s_ = i * 2 + jp
                        for chk in range(2):
                            s0 = chk * 512
                            pb = ps_pb.tile([128, 512], F32, tag="pb")
                            nc.tensor.matmul(
                                pb[:], OdP_s[:, s_ * 128:(s_ + 1) * 128],
                                P16[0:48, s0:s0 + 512],
                                start=True, stop=True)
                            dst = acc[:, jp * ST + s0:jp * ST + s0 + 512]
                            if d == 0:
                                # chunk = 2 n-blocks x 256 w
                                vi = vsv[:, 2 * chk:2 * chk + 2, i, :]
                                pbv = pb[:, :].rearrange(
                                    "p (n w) -> p n w", w=W)
                                dv = dst.rearrange("p (n w) -> p n w", w=W)
                            else:
                                vi = vsv[:, i, s0:s0 + 512]
                                pbv = pb[:, :]
                                dv = dst
                            if i == 0:
                                nc.vector.tensor_mul(dv, pbv, vi)
                            else:
                                p2 = p2p.tile([128, 512], BF16, tag="p2")
                                p2v = (p2[:, :].rearrange(
                                    "p (n w) -> p n w", w=W) if d == 0
                                    else p2[:, :])
                                nc.vector.tensor_mul(p2v, pbv, vi)
                                eng = nc.gpsimd if i in (1, 2) else nc.vector
                                eng.tensor_add(dst, dst, p2[:, :])

            # ---- combine: PE sums accH+accV -> psum; x added on the
            # psum->sbuf hop; DMA out. 512-pix chunks = 2 rows (same jr
            # half-pair, so one aH matmul each).
            accH, accV = accs
            aH = accH[:, :].rearrange("p (e n w) -> p e n w", e=2, w=W)
            aV = accV[:, :].rearrange("p (e h nb) -> p e h nb", e=2, nb=NB)
            o_t = xp.tile([64, PT], F32, tag="ot")
            for oc in range(PT // 512):
                o_ps = ps_o.tile([64, 512], F32, tag="o")
                rbase = 2 * oc
                jr0 = rbase % 4
                hb = 64 * (jr0 >> 1)
                n = rbase // 4
                stH = IdLo_s if hb == 0 else IdHi_s
                nc.tensor.matmul(o_ps[:], stH[:],
                                 aH[:, :, n, :],
                                 start=True, stop=False)
                ojc = o_ps[:, :].rearrange("p (r nb jc) -> p r nb jc",
                                           nb=NB, jc=A_)
                for jc in range(A_):
                    stV = IdLo_s if jc < 2 else IdHi_s
                    src = aV[:, jc & 1, rbase:rbase + 2, :]
                    nc.tensor.matmul(ojc[:, :, :, jc], stV[:],
                                     src, start=False, stop=(jc == A_ - 1))
                osl = o_t[:, oc * 512:(oc + 1) * 512]
                nc.vector.tensor_add(osl, o_ps[:],
                                     xr[:, oc * 512:(oc + 1) * 512]
                                     .bitcast(F32))
            nc.scalar.dma_start(out_d[:, r0:r0 + ROWS_T, :], o_t[:])

    nc.compile()
    return nc


def _run(x, Wq, bq, Wk, bk, Wv, bv, gamma, **spmd_kwargs):
    x = np.asarray(x, np.float32)
    WkR, Wq32, bq64, WvD, bvD, Osel, Osum, OdP, Id64 = _consts(
        np.asarray(Wq, np.float32), np.asarray(bq, np.float32),
        np.asarray(Wk, np.float32), np.asarray(bk, np.float32),
        np.asarray(Wv, np.float32), np.asarray(bv, np.float32),
        np.asarray(gamma, np.float32))

    if "nc" not in _cache:
        _cache["nc"] = _build()
    nc = _cache["nc"]

    in_maps = []
    for b in range(N_CORES):
        in_maps.append({"x": np.ascontiguousarray(x[b]), "WkR": WkR,
                        "Wq32": Wq32, "bq64": bq64, "WvD": WvD, "bvD": bvD,
                        "Osel": Osel, "Osum": Osum, "OdP": OdP, "Id64": Id64})
    res = run_bass_kernel_spmd(nc, in_maps, core_ids=list(range(N_CORES)),
                               **spmd_kwargs)
    out = np.stack([res.results[b]["out"] for b in range(N_CORES)], axis=0)
    return out, res


def kernel(x, Wq, bq, Wk, bk, Wv, bv, gamma):
    return _run(x, Wq, bq, Wk, bk, Wv, bv, gamma)[0]
